# revision 33
# baseline (speedup 1.0000x reference)
"""Trainium2 Bass kernel for DirCFConv-style GNN message passing, v4.

Computes, for inputs s:(B,N,H) f32, ef_mask:(B,N,N,H) f32, W:(H,H), b:(H,):
    m   = SiLU(LayerNorm(s @ W.T + b))          # (B,N,H)
    out[b,i,h] = sum_j ef_mask[b,i,j,h] * m[b,j,h]

Sharding: 8 cores, core c handles batch b = c // 2 and query-node half
i in [ (c%2)*256, (c%2)*256+256 ).  Each core streams its 64 MiB mask
shard from HBM (the roofline: 64 MiB at the 358 GB/s per-core cap is
187 us; v2's stream already ran at that rate from t=8.6..196.5us).

v2 layout (kept): SBUF partition p holds the JJ=4 consecutive j's
{4p..4p+3}, so each DMA descriptor covers a contiguous (jj,h) run of
2 KiB.  The multiply uses a partition-permuted m broadcast along i
with a 0-stride AP; the j-reduction is JJ accumulating PE matmuls per
query node on a bf16 product.

Measured mechanics this version is built around (baseline v2 222.8us;
best observed here 203.0us, but runs land bimodally at ~203 or ~224 --
the slow mode tracks chip-level HBM contention outside our control):
 - HWDGE descriptor generation runs ON the SP/ACT engines (DIRECT2D
   slices) and PARKS the engine in-trigger when the ~2 MiB descriptor
   ring is full.  Chunk size is therefore the flow-control quantum
   (1 MiB quarters), and any compute placed on SP/ACT delays the
   stream while any deep trigger prefetch delays that engine's
   compute.  Two queues sustain ~368 GB/s; one alone ~230.
 - DVE tensor_mul runs 2.2us/MiB when reading a tile the DMA has
   left, but stretches ~20% when racing concurrent DMA writes into
   the same 32KiB/partition tile region, and ~2x if the Pool engine
   runs concurrent multiplies (SBUF contention; Pool offload measured
   265us).  A dummy [P,1] copy off the NEXT tile's same chunk holds
   each multiply back one tile so it never races (tiles 2..9 only:
   lagging the late tiles rode the lag tail on top of slow-stream
   runs, regressing contended-chip runs to ~230 -- the last tiles are
   consumed the moment they land instead).
 - s/W/b loads sit at the HWDGE queue heads (SWDGE software
   descriptors cost ~1us each; they used to hold m back to t=33).
   Stage 1 computes hT = W @ sT + b 1^T with ONE wT-stationary matmul
   then PE-transposes back (f32 LDWEIGHTS is 2-pass, the old 8-matmul
   chain cost ~8us), and the LN tail fuses normalize+SiLU into one
   ACT op per block: m = silu(xc * rstd) via the scale operand.
 - Epilogues run mid-stream on DVE+PE only (block 0 after tile 7,
   i 128..239 after tile 14); all stores drain at the end on the two
   HWDGE queues, so only the last 16 i's epilogue (~1.5us) follows
   the final DMA.  The last tile's DMA tapers [4,4,2,2,2,2] so the
   final multiply covers 2 i's.
"""

import numpy as np

import concourse.bass as bass
import concourse.bacc as bacc
import concourse.tile as tile
from concourse import mybir
from concourse.bass_utils import run_bass_kernel_spmd
from concourse.masks import make_identity

B, N, H = 4, 512, 128
P = 128
JJ = N // P           # 4 consecutive j's per partition
ISUB = 16             # i's per mask tile -> 4 MiB DMAs
IH = N // 2           # 256 i's per core
N_CORES = 8
LN_EPS = 1e-5
F32 = mybir.dt.float32
BF16 = mybir.dt.bfloat16


def build_nc(ih=IH):
    nc = bacc.Bacc()
    s_d = nc.declare_dram_parameter("s", [N, H], F32, isOutput=False)
    w_d = nc.declare_dram_parameter("w", [H, H], F32, isOutput=False)
    b_d = nc.declare_dram_parameter("b", [H], F32, isOutput=False)
    mask_d = nc.declare_dram_parameter("mask", [ih, N, H], F32, isOutput=False)
    out_d = nc.declare_dram_parameter("out", [ih, H], F32, isOutput=True)

    nit = ih // ISUB
    # Full HW config gets the hand-tuned mid-stream epilogues; other ih
    # (CoreSim runs) use the generic end epilogue.
    full = ih == 2 * P and nit == 16

    with tile.TileContext(nc) as tc:
        with (
            tc.tile_pool(name="consts", bufs=1) as consts,
            tc.tile_pool(name="small", bufs=4) as small,
            tc.tile_pool(name="loads", bufs=4) as loads,
            tc.tile_pool(name="prods", bufs=3) as prods,
            tc.tile_pool(name="outs", bufs=3) as outs,
        ):
            stage1_psum = tc.tile_pool(name="spsum", bufs=1, space="PSUM")
            spsum = stage1_psum.__enter__()
            # ---------------- constants ----------------
            # gpsimd-produced constants all precede make_identity so the
            # single carrier wait (Pool sem) covers every one of them.
            ones_col = consts.tile([P, 1], BF16)
            nc.gpsimd.memset(ones_col, 1.0)
            ones_row = consts.tile([1, JJ * P], F32)
            nc.gpsimd.memset(ones_row, 1.0)
            # eps on DVE: its consumer (ACT Sqrt) already waits on DVE
            # for mv, and one DVE sem wait covers both.
            eps_t = consts.tile([P, 1], F32)
            nc.vector.memset(eps_t, LN_EPS)
            ident = consts.tile([P, P], F32)
            make_identity(nc, ident)

            # ---- parameter loads, at the HWDGE queue heads ----
            # w/b/s4 all load via the ACT queue: its first mask chunk
            # only fires at t~12 anyway, while putting anything before
            # sync's first mask trigger delays the whole stream start.
            w_sb = consts.tile([H, H], F32)
            nc.scalar.dma_start(out=w_sb, in_=w_d[:, :])
            bias_sb = consts.tile([1, H], F32)
            b_ap = b_d[:]
            bias_src = bass.AP(
                tensor=b_ap.tensor, offset=b_ap.offset, ap=[[0, 1]] + list(b_ap.ap)
            )
            nc.scalar.dma_start(out=bias_sb, in_=bias_src)
            # ONE contiguous s load: linear DRAM->SBUF maps partition p
            # to rows {4p..4p+3}, which IS the j = JJ*p + jj layout the
            # m_perm convention needs; block jj is just the free-axis
            # slice s4[:, jj, :].  (The old 4 strided gathers cost 512
            # descriptors of 512B at the queue heads.)
            s4 = consts.tile([P, JJ, H], F32)
            nc.scalar.dma_start(
                out=s4, in_=s_d[:, :].rearrange("(p jj) h -> p jj h", jj=JJ)
            )
            s_sbs = [s4[:, jj, :] for jj in range(JJ)]

            # Wait-carrier: walrus allows only ONE sync wait per
            # Matmult, so absorb the gpsimd(identity) dependency into a
            # throwaway PE op; later matmuls then only carry their own
            # single DMA/engine wait.
            carrier_ps = spsum.tile([P, P], F32)
            nc.tensor.transpose(carrier_ps, ident, ident)

            # Dummy Sqrt to prefetch the ACT function table at t~7.5
            # (ACT is otherwise idle there); the real LN sqrts then
            # skip the 1.3us ACT_TABLE_LOAD on the m critical path.
            warm = small.tile([1, 1], F32, tag="warm")
            nc.scalar.activation(
                warm, eps_t[0:1, 0:1], mybir.ActivationFunctionType.Sqrt
            )

            # W^T via PE-transpose: (o,h) -> (h,o)
            wT_ps = spsum.tile([H, H], F32)
            nc.tensor.transpose(wT_ps, w_sb, ident)
            wT_sb = consts.tile([H, H], F32)
            nc.scalar.copy(wT_sb, wT_ps)

            # ------------- m = SiLU(LN(s @ W.T + b)) -------------
            # hT[o, j] = sum_h W[o,h] sT[h,j] + b[o]: ONE wT-stationary
            # matmul over 512 moving columns instead of eight
            # sT-stationary f32 matmuls (f32 LDWEIGHTS is 2-pass at 4x
            # row cost; the old chain held m back to t=28, an ~8us DVE
            # deficit that persisted to the end of the stream).  Then
            # transpose hT back to [j, o] blocks for the free-axis LN.
            sT_all = spsum.tile([P, JJ * P], F32)
            h_all = spsum.tile([P, JJ * H], F32)
            hT_ps = spsum.tile([P, JJ * P], F32)
            for jj in range(JJ):
                nc.tensor.matmul(
                    sT_all[:, jj * P:(jj + 1) * P],
                    lhsT=s_sbs[jj],
                    rhs=ident,
                    is_transpose=True,
                    start=(jj == 0),
                    stop=(jj == JJ - 1),
                )
            sT_sb = consts.tile([P, JJ * P], F32)
            nc.scalar.copy(sT_sb, sT_all)
            nc.tensor.matmul(
                hT_ps, lhsT=wT_sb, rhs=sT_sb, start=True, stop=False
            )
            nc.tensor.matmul(
                hT_ps, lhsT=bias_sb, rhs=ones_row, start=False, stop=True
            )
            hT_sb = consts.tile([P, JJ * P], F32)
            nc.vector.tensor_copy(hT_sb, hT_ps)
            for jj in range(JJ):
                nc.tensor.matmul(
                    h_all[:, jj * H:(jj + 1) * H],
                    lhsT=hT_sb[:, jj * P:(jj + 1) * P],
                    rhs=ident,
                    is_transpose=True,
                    start=(jj == 0),
                    stop=(jj == JJ - 1),
                )

            # ---- mask prefetch: tile 0's triggers ----
            # Emitted here (after the sT/hT chain, before the LN tail)
            # so the ACT engine's first mask triggers precede its
            # sqrt/silu ops in program order: ACT then generates
            # descriptors during t~12-25 instead of idling on DVE stats
            # while the sync queue streams alone at ~230 GB/s.  Only
            # ONE tile: hoisting two parked ACT inside the ~2 MiB
            # descriptor ring until t~35, delaying sqrt/silu and m.
            PF = min(2, nit)
            mts = {}

            def tile_chunks(it):
                # Last tile tapers so the final multiply (on the
                # critical path after the final DMA) covers only 2 i's.
                if it == nit - 1 and ISUB == 16 and nit > 1:
                    return [4, 4, 2, 2, 2, 2]
                return [4, 4, 4, 4] if nit > 1 else [ISUB]

            def issue_tile_dma(it):
                mt = loads.tile([P, ISUB, JJ, H], F32, tag="mt", name=f"mt{it}")
                mts[it] = mt
                off = 0
                for ci, sz in enumerate(tile_chunks(it)):
                    src = mask_d[
                        it * ISUB + off:it * ISUB + off + sz, :, :
                    ].rearrange("i (p jj) h -> p i jj h", jj=JJ)
                    deng = nc.sync if (it + ci) % 2 == 0 else nc.scalar
                    deng.dma_start(out=mt[:, off:off + sz], in_=src)
                    off += sz

            issue_tile_dma(0)

            # m_perm[p, jj, :] = m[JJ*p + jj, :]
            # Sqrt and Sigmoid phases in separate jj-loops: alternating
            # them per jj makes ACT reload its 16 KiB function table
            # (1.3us each), serializing the m_perm critical path.
            m_perm = consts.tile([P, JJ, H], F32)
            xcs, stdvs = [], []
            for jj in range(JJ):
                h_ps = h_all[:, jj * H:(jj + 1) * H]
                stats = small.tile([P, 6], F32, tag=f"stats{jj}")
                nc.vector.bn_stats(stats, h_ps)
                mv = small.tile([P, 2], F32, tag=f"mv{jj}")
                nc.vector.bn_aggr(mv, stats)
                xc = small.tile([P, H], F32, tag=f"xc{jj}")
                nc.vector.tensor_scalar_sub(xc, h_ps, mv[:, 0:1])
                stdv = small.tile([P, 1], F32, tag=f"stdv{jj}")
                nc.scalar.activation(
                    stdv, mv[:, 1:2], mybir.ActivationFunctionType.Sqrt, bias=eps_t
                )
                xcs.append(xc)
                stdvs.append(stdv)
            rstds = []
            for jj in range(JJ):
                rstd = small.tile([P, 1], F32, tag=f"rstd{jj}")
                nc.vector.reciprocal(rstd, stdvs[jj])
                rstds.append(rstd)
            for jj in range(JJ):
                # One fused ACT op: m = silu(xc * rstd).  The old
                # sigmoid + two DVE muls cost ~3us of ACT<->DVE
                # ping-pong on the m critical path.
                nc.scalar.activation(
                    m_perm[:, jj, :], xcs[jj],
                    mybir.ActivationFunctionType.Silu, scale=rstds[jj]
                )
            if PF > 1:
                issue_tile_dma(1)

            # m broadcast along the i axis: 0-stride free axis.
            def m_bcast(n):
                return bass.AP(
                    tensor=m_perm.tensor,
                    offset=m_perm.offset,
                    ap=[list(m_perm.ap[0]), [0, n]]
                    + [list(x) for x in m_perm.ap[1:]],
                )

            # stage-1 PSUM pools stay open: releasing them would put a
            # (PE+DVE) release-wait on stage-2's first Matmult, which
            # walrus cannot encode.
            # ------- out[i,h] = sum_j mask[i,j,h] * m[j,h] -------
            # acc2[h, i] += pt[:, ii, jj, :].T @ ones  (partition-reduce
            # over p via PE, free-axis reduce over jj via PSUM accum).
            opsum_cm = tc.tile_pool(name="opsum", bufs=1, space="PSUM")
            opsum = opsum_cm.__enter__()
            tpsum_cm = tc.tile_pool(name="tpsum", bufs=2, space="PSUM")
            tpsum = tpsum_cm.__enter__()
            acc2 = opsum.tile([P, ih], F32)

            def epi_compute(tag, i0, w):
                """acc2[:, i0:i0+w] -> oT [w, H] in SBUF (DVE+PE only)."""
                accT = outs.tile([P, w], F32, tag="accT", name=f"accT{tag}",
                                 bufs=2)
                nc.vector.tensor_copy(accT, acc2[:, i0:i0 + w])
                tp = tpsum.tile([w, P], F32, tag="tp", name=f"tp{tag}")
                nc.tensor.transpose(tp, accT, ident)
                oT = outs.tile([w, P], F32, tag="oT", name=f"oT{tag}", bufs=2)
                nc.vector.tensor_copy(oT, tp)
                return oT

            oT0 = oTA = None
            for it in range(nit):
                # 1 MiB quarters on alternating HWDGE queues:
                # descriptor generation runs ON the SP/ACT engines
                # (DIRECT2D) and parks in-trigger under ring
                # backpressure, so the chunk size is the flow-control
                # quantum -- finer quanta rebalance the queues faster
                # than 2 MiB halves.
                nxt = max(it + 1, PF)
                if nxt < nit and nxt not in mts:
                    issue_tile_dma(nxt)
                mt = mts[it]
                off = 0
                for ci, sz in enumerate(tile_chunks(it)):
                    # Deliberate 1-tile consumer lag: DVE multiplies run
                    # at 2.2us/MiB when reading a tile the DMA has left,
                    # but stretch ~20% when racing concurrent DMA writes
                    # into the same 32KiB/partition tile region.  A
                    # dummy [P,1] copy off the NEXT tile's same chunk
                    # delays this multiply until that chunk landed, so
                    # the real multiply never races.  The last 3 tiles
                    # stay lag-free to keep the post-stream tail short.
                    if 2 <= it and it + 1 < nit - 6 and ci < 4:
                        dly = small.tile([P, 1], F32, tag="dly",
                                         name=f"dly{it}_{ci}", bufs=2)
                        nc.vector.tensor_copy(
                            dly, mts[it + 1][:, off, 0, 0:1]
                        )
                    pt = prods.tile([P, sz, JJ, H], BF16, tag=f"pt{sz}",
                                    name=f"pt{it}_{ci}",
                                    bufs=3)
                    # The multiply stays on DVE alone: offloading
                    # alternate chunks to the Pool engine stretched BOTH
                    # engines ~2x (SBUF bandwidth contention, measured
                    # 265us), so concurrency there is a net loss.
                    nc.vector.tensor_mul(
                        pt, mt[:, off:off + sz], m_bcast(sz)
                    )
                    for ii in range(sz):
                        i = it * ISUB + off + ii
                        for jj in range(JJ):
                            # One accumulation group spans the whole
                            # bank: start zeroes the full zero region,
                            # so only the global first/last matmuls
                            # carry start/stop.
                            nc.tensor.matmul(
                                acc2[:, i:i + 1],
                                lhsT=pt[:, ii, jj, :],
                                rhs=ones_col,
                                start=(i == 0 and jj == 0),
                                stop=(i == ih - 1 and jj == JJ - 1),
                            )
                    # i 240..253 fully accumulated after chunk 4 of the
                    # last tile: epilogue off the critical path so only
                    # the last 2 rows follow the final DMA.
                    if full and it == nit - 1 and ci == 4:
                        oTB1 = epi_compute("B1", ih - ISUB, ISUB - 2)
                    off += sz
                # Block 0 (i 0..127) is fully accumulated after tile 7:
                # run its epilogue mid-stream using ONLY DVE + PE.
                if full and it == 7:
                    oT0 = epi_compute("0", 0, P)
                # i 128..239 fully accumulated after tile 14: compute
                # their epilogue off the critical path; store at end.
                if full and it == nit - 2:
                    oTA = epi_compute("A", P, ih - P - ISUB)

            if full:
                # blk0/A/B1 stores drain right after the last sync-queue
                # mask descriptors; B2 (the last 2 i's: one transpose +
                # a 1 KiB store) is the only post-stream epilogue.
                nc.sync.dma_start(out=out_d[0:P, :], in_=oT0)
                nc.sync.dma_start(out=out_d[P:ih - ISUB, :], in_=oTA)
                nc.sync.dma_start(out=out_d[ih - ISUB:ih - 2, :], in_=oTB1)
                oTB2 = epi_compute("B2", ih - 2, 2)
                nc.scalar.dma_start(out=out_d[ih - 2:ih, :], in_=oTB2)
            else:
                nblk = (ih + P - 1) // P
                for blk in range(nblk):
                    w = min(P, ih - blk * P)
                    oT = epi_compute(f"g{blk}", blk * P, w)
                    deng = nc.sync if blk % 2 == 0 else nc.scalar
                    deng.dma_start(out=out_d[blk * P:blk * P + w, :], in_=oT)
            tpsum_cm.__exit__(None, None, None)
            opsum_cm.__exit__(None, None, None)
            stage1_psum.__exit__(None, None, None)
    nc.finalize()
    return nc


_NC_CACHE = {}


def _get_nc():
    key = "main"
    if key not in _NC_CACHE:
        _NC_CACHE[key] = build_nc()
    return _NC_CACHE[key]


def kernel(s, ef_mask, W, b):
    s = np.ascontiguousarray(s, dtype=np.float32)
    ef_mask = np.ascontiguousarray(ef_mask, dtype=np.float32)
    W = np.ascontiguousarray(W, dtype=np.float32)
    b = np.ascontiguousarray(b, dtype=np.float32)

    nc = _get_nc()
    in_maps = []
    for c in range(N_CORES):
        bb = c // 2
        half = c % 2
        in_maps.append(
            {
                "s": s[bb],
                "w": W,
                "b": b,
                "mask": ef_mask[bb, half * IH:(half + 1) * IH],
            }
        )
    res = run_bass_kernel_spmd(nc, in_maps, list(range(N_CORES))).results
    out = np.empty((B, N, H), dtype=np.float32)
    for c in range(N_CORES):
        bb = c // 2
        half = c % 2
        out[bb, half * IH:(half + 1) * IH] = res[c]["out"]
    return out
